# revision 31
# baseline (speedup 1.0000x reference)
"""Trainium2 Bass kernel for EfficientAttention (linear attention block).

Computation (per batch b, head h):
    qkv = x @ w_qkv.T + b_qkv
    q = softmax(q, axis=head_dim) * head_dim**-0.5
    k = softmax(k, axis=seqlen)
    kv[d,e] = sum_s k[s,d] v[s,e]          (per-head 64x64 state)
    out[s,e] = sum_d q[s,d] kv[d,e]
    y = out @ w_proj.T + b_proj
Sharding: 8 cores = (batch b = c//2, seq half = c%2); 2048 tokens per core.
Cross-core coupling: kv state + k-softmax Z -> AllReduce (pairs) [128,520] bf16.

v3 design (vs the bf16 v2 baseline, 281.7us):
- k and q projections in fp8-e4m3 with DoubleRow (contraction 256/matmul,
  ~1.5-2x PE throughput). Weights pre-scaled x64 host-side (avoids e4m3
  subnormals); the 1/64 descale folds into the ACT exp (scale=1/64).
  Measured numerics (cpu sim): k+q fp8, rest f16 -> l2-rel 1.17e-2 (< 2e-2).
- v and out projections + all small matmuls in fp16 (same PE speed as bf16,
  8x lower quantization noise -> error budget goes to the fp8 paths).
- One resident fp8 x copy [128,2,T] per 256-dim chunk serves BOTH k-proj
  (stationary) and q-proj (moving): phase 2 has zero input DMA.
- Phase 2 is j-major with c (256-chunk) outer so the DR stationary
  wq8[c,j] is amortized over 4 moving matmuls (LDWEIGHTS 1:4).
- AllReduce payload in bf16 (halves the 27.5us fp32 collective).
- Everything gated on the collective result (kvred cast, kv row-normalize
  via normalize_recip, off-diag zero memsets) runs on GpSimd, which is
  otherwise idle: the v2 trace showed the kv reciprocal parked mid-DVE-FIFO
  stalling the whole q-sweep pipeline 8.7us behind the collective.
"""

import sys

sys.path.insert(0, "/opt/trn_rl_repo")

import numpy as np

import concourse.bacc as bacc
import concourse.tile as tile
from concourse import mybir
from concourse import bass_utils

F32 = mybir.dt.float32
F16 = mybir.dt.float16
BF16 = mybir.dt.bfloat16
F8 = mybir.dt.float8e4
DR = mybir.MatmulPerfMode.DoubleRow

D = 1024          # model dim (= qkv contraction dim)
T = 2048          # tokens per core (one batch element's half sequence)
NH = 16           # heads
HD = 64           # head dim
NPAIR = 8         # head pairs (2 heads / 128 partitions)
KC = D // 128     # contraction chunks of 128
NC2 = KC // 2     # fp8 DoubleRow chunks of 256
TB = T // 128     # token blocks of 128
SB = T // 512     # token column blocks of 512
SCALE = HD ** -0.5
WS = 64.0         # fp8 weight scale (k/q)

N_CORES = 8


def build_program(with_bias=False):
    nc = bacc.Bacc("TRN2", target_bir_lowering=False, num_devices=N_CORES)

    x8d = nc.dram_tensor("x8d", [NC2, 128, 2, T], F8, kind="ExternalInput")
    x16d = nc.dram_tensor("x16d", [TB, 128, KC, 128], F16, kind="ExternalInput")
    wk8d = nc.dram_tensor("wk8d", [NC2, 128, 2, D], F8, kind="ExternalInput")
    wq8d = nc.dram_tensor("wq8d", [NC2, 128, 2, D], F8, kind="ExternalInput")
    wv16d = nc.dram_tensor("wv16d", [KC, 128, D], F16, kind="ExternalInput")
    wp16d = nc.dram_tensor("wp16d", [KC, 128, D], F16, kind="ExternalInput")
    bq = nc.dram_tensor("bq", [D], F32, kind="ExternalInput")
    bk = nc.dram_tensor("bk", [D], F32, kind="ExternalInput")   # pre-scaled x64
    bv = nc.dram_tensor("bv", [D], F32, kind="ExternalInput")
    bp = nc.dram_tensor("bp", [D], F32, kind="ExternalInput")
    cst = nc.dram_tensor("cst", [128, 132], F16, kind="ExternalInput")
    y = nc.dram_tensor("y", [T, D], F32, kind="ExternalOutput")

    def bias_bcast(b):
        # DRAM [D] broadcast-load to SBUF [128, D] (partition step 0)
        import concourse.bass as bass
        ap = b[:]
        return bass.AP(tensor=ap.tensor, offset=ap.offset, ap=[[0, 128]] + list(ap.ap))

    with tile.TileContext(nc) as tc:
        with (
            tc.tile_pool(name="const", bufs=1) as const,
            tc.tile_pool(name="wpool", bufs=1) as wpool,
            tc.tile_pool(name="xpool", bufs=1) as xpool,
            tc.tile_pool(name="xin", bufs=3) as xin,
            tc.tile_pool(name="ekv", bufs=2) as ekv,
            tc.tile_pool(name="acc", bufs=1) as accp,
            tc.tile_pool(name="qpool", bufs=2) as qpool,
            tc.tile_pool(name="qt", bufs=1) as qtpool,
            tc.tile_pool(name="atn", bufs=2) as atnp,
            tc.tile_pool(name="kvsb", bufs=1) as kvsbp,
            tc.tile_pool(name="yout", bufs=2) as youtp,
            tc.tile_pool(name="psum", bufs=2, space="PSUM") as psum,
            tc.tile_pool(name="dram", bufs=1, space="DRAM") as dram,
        ):
            # warmup fodder: memset, no DMA dependency -> PE HAM warms ASAP
            warm = const.tile([128, 128], F16, tag="warm")
            nc.gpsimd.memset(warm, 0.25)
            cst_sb = const.tile([128, 132], F16, tag="cst")
            m8 = cst_sb[:, 0:128]

            if with_bias:
                bk_sb = const.tile([128, D], F16, tag="bk")
                bv_sb = const.tile([128, D], F16, tag="bv")
                bp_sb = const.tile([128, D], F16, tag="bp")
                nc.gpsimd.dma_start(bk_sb, bias_bcast(bk))
                nc.gpsimd.dma_start(bv_sb, bias_bcast(bv))
                nc.gpsimd.dma_start(bp_sb, bias_bcast(bp))
                bq_col = const.tile([128, KC], F32, tag="bqc")
                nc.gpsimd.dma_start(bq_col, bq[:].rearrange("(kc p) -> p kc", p=128))

            # weights + resident fp8 x, chunked so startup streams through
            # arriving chunks. gpsimd ring: wk8 -> wv16 -> wq8 -> wp16.
            # sync ring: cst -> x8 chunks -> x16 startup tiles -> steady x16.
            wk8c = [wpool.tile([128, 2, D], F8, tag=f"wk{c}", name=f"wk{c}")
                    for c in range(NC2)]
            wq8c = [wpool.tile([128, 2, D], F8, tag=f"wq{c}", name=f"wq{c}")
                    for c in range(NC2)]
            wv16c = [wpool.tile([128, D], F16, tag=f"wv{k}", name=f"wv{k}")
                     for k in range(KC)]
            wp16c = [wpool.tile([128, D], F16, tag=f"wp{k}", name=f"wp{k}")
                     for k in range(KC)]
            x8c = [xpool.tile([128, 2, T], F8, tag=f"x8{c}", name=f"x8{c}")
                   for c in range(NC2)]
            NSTART = 3  # tbs processed weight-chunk-major at startup
            xpre = [xin.tile([128, KC, 128], F16, tag="x", bufs=4,
                             name=f"xpre{t}")
                    for t in range(NSTART)]
            for c in range(NC2):
                nc.gpsimd.dma_start(wk8c[c], wk8d[c])
                # x8 chunks split across the sync and scalar rings so the
                # DMA-paced k-startup gets ~2x input bandwidth
                if c % 2 == 0:
                    nc.sync.dma_start(x8c[c], x8d[c])
                else:
                    nc.scalar.dma_start(x8c[c], x8d[c])
            for t in range(NSTART):
                nc.sync.dma_start(xpre[t], x16d[t])
            nc.sync.dma_start(cst_sb, cst[:])
            for k in range(KC):
                # wv16 split gpsimd/vector rings (v-startup needs it early)
                if k % 2 == 0:
                    nc.gpsimd.dma_start(wv16c[k], wv16d[k])
                else:
                    nc.scalar.dma_start(wv16c[k], wv16d[k])
            # dummy normalize_recip preloads the gpsimd custom-op library
            # (LOAD_LIB + ~13us engine DRAIN) so the real calls after the
            # collective run in ~300ns; wq/wp (needed ~100us later) queue
            # behind the drain.
            nr_in = const.tile([128, 17], F32, tag="nr_in")
            nr_out = const.tile([128, 16], F16, tag="nr_out")
            nc.gpsimd.memset(nr_in, 1.0)
            nc.gpsimd.normalize_recip(nr_out, nr_in[:, 0:16], nr_in[:, 16:17])
            for c in range(NC2):
                nc.gpsimd.dma_start(wq8c[c], wq8d[c])
            for k in range(KC):
                nc.gpsimd.dma_start(wp16c[k], wp16d[k])

            # PE warmup on the memset tile: HAM busy-window fills while the
            # first weight/x chunks are still in flight.
            for w in range(24):
                wps = psum.tile([128, 512], F32, tag="f", bufs=1)
                nc.tensor.matmul(wps[:, 0:128], warm, warm, start=True, stop=True)

            # ---- Phase 1 startup: tbs 0..3 chunk-major so each arriving
            # wk8/x8 chunk feeds 8 DR matmuls immediately.
            tag8 = ("a", "a", "b", "b", "c", "d", "e", "f")[:2 * NSTART]

            ek_s = [ekv.tile([128, D], F16, tag="ek", bufs=4, name=f"eks{t}")
                    for t in range(NSTART)]
            # vv layout [128, pair, 130]: cols 0:128 = v, col 128 = 1.0 (the
            # kv matmul's 129th output column catches Z), col 129 pad.
            vv_s = [ekv.tile([128, NPAIR, 130], F16, tag="v", bufs=3,
                             name=f"vvs{t}")
                    for t in range(NSTART)]
            for t in range(NSTART):
                nc.vector.memset(vv_s[t][:, :, 128:129], 1.0)

            ktiles = [psum.tile([128, 512], F32, tag=tag8[i],
                                bufs=(2 if tag8[i] in ("a", "b") else 1),
                                name=f"spk{i}")
                      for i in range(2 * NSTART)]
            for c in range(NC2):
                for i in range(2 * NSTART):
                    t, half = divmod(i, 2)
                    nc.tensor.matmul(
                        ktiles[i], x8c[c][:, :, t * 128:(t + 1) * 128],
                        wk8c[c][:, :, half * 512:(half + 1) * 512],
                        start=(c == 0), stop=(c == NC2 - 1), perf_mode=DR)
            for i in range(2 * NSTART):
                t, half = divmod(i, 2)
                sl = slice(half * 512, (half + 1) * 512)
                if with_bias:
                    nc.vector.tensor_add(ktiles[i], ktiles[i], bk_sb[:, sl])
                nc.scalar.activation(ek_s[t][:, sl], ktiles[i],
                                     mybir.ActivationFunctionType.Exp,
                                     scale=1.0 / WS)
            vtiles = [psum.tile([128, 512], F32, tag=tag8[i],
                                bufs=(2 if tag8[i] in ("a", "b") else 1),
                                name=f"spv{i}")
                      for i in range(2 * NSTART)]
            for kc in range(KC):
                for i in range(2 * NSTART):
                    t, half = divmod(i, 2)
                    nc.tensor.matmul(
                        vtiles[i], xpre[t][:, kc, :],
                        wv16c[kc][:, half * 512:(half + 1) * 512],
                        start=(kc == 0), stop=(kc == KC - 1))
            for i in range(2 * NSTART):
                t, half = divmod(i, 2)
                sl = slice(half * 512, (half + 1) * 512)
                if with_bias:
                    nc.vector.tensor_add(vtiles[i], vtiles[i], bv_sb[:, sl])
                nc.scalar.copy(vv_s[t][:, 4 * half:4 * half + 4, 0:128],
                               vtiles[i])

            # kv accumulators: 3 pairs per bank at 129-wide slots -- col 128
            # of each slot catches Z (the vv moving operand carries a ones
            # column), killing the 128 FD=1 zk matmuls of v2.
            kvps = [psum.tile([128, 512], F32, tag=t, name=f"kv{t}", bufs=1)
                    for t in ("c", "d", "e")]

            def emit_kvzk(tb, ek, vv):
                for p in range(NPAIR):
                    bank, pp = divmod(p, 3)
                    ekp = ek[:, p * 128:(p + 1) * 128]
                    nc.tensor.matmul(
                        kvps[bank][:, pp * 129:pp * 129 + 129],
                        ekp, vv[:, p, 0:129],
                        start=(tb == 0 and pp == 0),
                        stop=(tb == TB - 1 and pp == min(2, NPAIR - 1 - bank * 3)))

            for t in range(NSTART):
                emit_kvzk(t, ek_s[t], vv_s[t])

            # ---- Phase 1 steady state: remaining tbs one at a time ----
            for tb in range(NSTART, TB):
                xtile = xin.tile([128, KC, 128], F16, tag="x", bufs=4)
                nc.sync.dma_start(xtile, x16d[tb])
                ek = ekv.tile([128, D], F16, tag="ek", bufs=4)
                vv = ekv.tile([128, NPAIR, 130], F16, tag="v", bufs=3)
                nc.vector.memset(vv[:, :, 128:129], 1.0)
                psk = [psum.tile([128, 512], F32, tag="a", bufs=2,
                                 name=f"psk{tb}_{h2}")
                       for h2 in range(2)]
                for c in range(NC2):
                    for half in range(2):
                        nc.tensor.matmul(
                            psk[half], x8c[c][:, :, tb * 128:(tb + 1) * 128],
                            wk8c[c][:, :, half * 512:(half + 1) * 512],
                            start=(c == 0), stop=(c == NC2 - 1), perf_mode=DR)
                for half in range(2):
                    sl = slice(half * 512, (half + 1) * 512)
                    if with_bias:
                        nc.vector.tensor_add(psk[half], psk[half], bk_sb[:, sl])
                    # h1 exp in 128-col pieces: kv matmuls consume per-pair
                    # columns, finer ACT ops unblock them sooner.
                    nsp = 1 if half == 0 else 4
                    for i in range(nsp):
                        w = 512 // nsp
                        nc.scalar.activation(
                            ek[:, half * 512 + i * w:half * 512 + (i + 1) * w],
                            psk[half][:, i * w:(i + 1) * w],
                            mybir.ActivationFunctionType.Exp, scale=1.0 / WS)
                for half in range(2):
                    sl = slice(half * 512, (half + 1) * 512)
                    psv = psum.tile([128, 512], F32, tag="b", bufs=2)
                    for kc in range(KC):
                        nc.tensor.matmul(psv, xtile[:, kc, :],
                                         wv16c[kc][:, sl],
                                         start=(kc == 0), stop=(kc == KC - 1))
                    if with_bias:
                        nc.vector.tensor_add(psv, psv, bv_sb[:, sl])
                    nsp = 2 if half == 0 else 4
                    for i in range(nsp):
                        w = 512 // nsp
                        npr = w // 128
                        nc.scalar.copy(
                            vv[:, 4 * half + i * npr:4 * half + (i + 1) * npr,
                               0:128],
                            psv[:, i * w:(i + 1) * w])
                emit_kvzk(tb, ek, vv)

            # ---- stage compacted partial (kv | Z) in bf16, AllReduce pairs.
            # pair p -> cols [64p : 64p+64]; head A rows 0:64, head B 64:128
            stage = accp.tile([128, 520], BF16, tag="stage")
            for p in range(NPAIR):
                bank, pp = divmod(p, 3)
                base = pp * 129
                nc.vector.tensor_copy(stage[0:64, 64 * p:64 * p + 64],
                                      kvps[bank][0:64, base:base + 64])
                nc.vector.tensor_copy(stage[64:128, 64 * p:64 * p + 64],
                                      kvps[bank][64:128, base + 64:base + 128])
                nc.vector.tensor_copy(stage[:, 512 + p:513 + p],
                                      kvps[bank][:, base + 128:base + 129])
            cin = dram.tile([128, 520], BF16, tag="cin")
            cout = dram.tile([128, 520], BF16, tag="cout")
            nc.gpsimd.dma_start(cin, stage)
            nc.gpsimd.collective_compute(
                "AllReduce", mybir.AluOpType.add,
                replica_groups=[[0, 1], [2, 3], [4, 5], [6, 7]],
                ins=[cin[:].opt()], outs=[cout[:].opt()])
            kvred = accp.tile([128, 520], BF16, tag="kvred")
            nc.gpsimd.dma_start(kvred, cout)
            # compact per-pair kv [128 dims, 64 out-dims]: row r's Z sits in
            # kvred[r, 512+p], so one full-partition normalize_recip per pair
            # does the whole row-normalize. All on GpSimd (lib preloaded at
            # t=0): nothing in the DVE/ACT/PE FIFOs may wait on the
            # collective, or the q-sweep pipeline stalls behind it.
            kvredf = accp.tile([128, 520], F32, tag="kvredf")
            nc.gpsimd.tensor_copy(kvredf, kvred)
            kv_sb = [kvsbp.tile([128, 64], F16, tag=f"kv{p}", name=f"kv{p}")
                     for p in range(NPAIR)]
            for p in range(NPAIR):
                nc.gpsimd.normalize_recip(
                    kv_sb[p], kvredf[:, 64 * p:64 * p + 64],
                    kvredf[:, 512 + p:513 + p])

            # ---- Phase 2: q projection, fp8 DR, j-major / c-outer so the
            # stationary wq8[c, j*128:+128] covers 4 moving matmuls.
            # exp on ACT (scale 1/64) -> f16; Z via m8 mask matmul; recip on
            # DVE; qtall = equ * rr. z-flushes defer one c-iteration so the
            # PE never waits on ACT.
            # qtall holds UNNORMALIZED exp(q); the 1/(8Zq) factor (rrall,
            # f32, one slot per (j, sb)) is applied by the attention-phase
            # PSUM->SBUF copy, which becomes a multiply for free.
            qtall = qtpool.tile([128, NPAIR, T], F16, tag="qtall")
            rrall = qtpool.tile([128, NPAIR * SB, 512], F32, tag="rrall")
            pend = []
            zq_n = [0]

            def flush_z(item):
                j, sb = item
                zq = psum.tile([128, 512], F32, tag=("e", "f")[zq_n[0] % 2],
                               bufs=1)
                zq_n[0] += 1
                nc.tensor.matmul(zq, m8,
                                 qtall[:, j, sb * 512:(sb + 1) * 512],
                                 start=True, stop=True)
                nc.vector.reciprocal_approx_fast(rrall[:, j * SB + sb, :], zq)

            qtag4 = ("a", "a", "b", "b")
            for j in range(NPAIR):
                psq = [psum.tile([128, 512], F32, tag=qtag4[sb], bufs=2,
                                 name=f"q{j}_{sb}")
                       for sb in range(SB)]
                for c in range(NC2):
                    for sb in range(SB):
                        nc.tensor.matmul(
                            psq[sb], wq8c[c][:, :, j * 128:(j + 1) * 128],
                            x8c[c][:, :, sb * 512:(sb + 1) * 512],
                            start=(c == 0), stop=(c == NC2 - 1), perf_mode=DR)
                    if pend:
                        flush_z(pend.pop(0))
                for sb in range(SB):
                    if with_bias:
                        nc.scalar.activation(qtall[:, j, sb * 512:(sb + 1) * 512],
                                             psq[sb],
                                             mybir.ActivationFunctionType.Exp,
                                             scale=1.0 / WS,
                                             bias=bq_col[:, j:j + 1])
                    else:
                        nc.scalar.activation(qtall[:, j, sb * 512:(sb + 1) * 512],
                                             psq[sb],
                                             mybir.ActivationFunctionType.Exp,
                                             scale=1.0 / WS)
                    pend.append((j, sb))
            while pend:
                flush_z(pend.pop(0))

            # ---- attention + output projection, per 512-token block.
            # attn runs one sb ahead of outproj so the PSUM->SBUF copies
            # complete before outproj consumes them.
            at_tiles = {}

            def emit_attn(sb):
                at = atnp.tile([128, NPAIR, 512], F16, tag="at")
                at_tiles[sb] = at
                for p in range(NPAIR):
                    aps = psum.tile([128, 512], F32, tag=("c", "d")[p % 2],
                                    bufs=1)
                    # two 64x64 head matmuls packed at array tiles (0,0) and
                    # (64,64): concurrent in the PE, replaces the block-diag
                    # 128x128 stationary.
                    nc.tensor.matmul(aps[0:64, :], kv_sb[p][0:64, :],
                                     qtall[0:64, p, sb * 512:(sb + 1) * 512],
                                     start=True, stop=True)
                    nc.tensor.matmul(aps[64:128, :], kv_sb[p][64:128, :],
                                     qtall[64:128, p, sb * 512:(sb + 1) * 512],
                                     start=True, stop=True,
                                     tile_position=(64, 64))
                    nc.vector.tensor_mul(at[:, p, :], aps,
                                         rrall[:, p * SB + sb, :])

            def emit_outproj(sb):
                at = at_tiles.pop(sb)
                for tb2 in range(4):
                    for oc in range(2):
                        sl = slice(oc * 512, (oc + 1) * 512)
                        ps = psum.tile([128, 512], F32, tag="b")
                        for kc in range(KC):
                            nc.tensor.matmul(
                                ps, at[:, kc, tb2 * 128:(tb2 + 1) * 128],
                                wp16c[kc][:, sl],
                                start=(kc == 0), stop=(kc == KC - 1))
                        yt = youtp.tile([128, 512], F32, tag="y")
                        if with_bias:
                            nc.vector.tensor_add(yt, ps, bp_sb[:, sl])
                        else:
                            nc.scalar.copy(yt, ps)
                        nc.sync.dma_start(
                            y[(sb * 4 + tb2) * 128:(sb * 4 + tb2 + 1) * 128, sl],
                            yt)

            emit_attn(0)
            for sb in range(SB):
                if sb + 1 < SB:
                    emit_attn(sb + 1)
                emit_outproj(sb)

    nc.compile()
    return nc


_NC = {}


def _get_nc(with_bias=False):
    if with_bias not in _NC:
        _NC[with_bias] = build_program(with_bias=with_bias)
    return _NC[with_bias]


def kernel(x, w_qkv, b_qkv, w_proj, b_proj):
    import ml_dtypes

    f8 = ml_dtypes.float8_e4m3
    f16 = np.float16
    x = np.asarray(x, dtype=np.float32)
    w_qkv = np.asarray(w_qkv, dtype=np.float32)
    b_qkv = np.asarray(b_qkv, dtype=np.float32)
    w_proj = np.asarray(w_proj, dtype=np.float32)
    b_proj = np.asarray(b_proj, dtype=np.float32)

    bs, seqlen, dim = x.shape
    half = seqlen // 2

    def to8(a):
        return np.clip(a, -240, 240).astype(f8)

    # weights: [in-dim, out-dim] layouts; k/q fp8 chunked [NC2,128,2,D],
    # v/p f16 chunked [KC,128,D]
    wqT = np.ascontiguousarray(w_qkv[0:D].T)
    wkT = np.ascontiguousarray(w_qkv[D:2 * D].T)
    wvT = np.ascontiguousarray(w_qkv[2 * D:3 * D].T)
    wpT = np.ascontiguousarray(w_proj.T)
    wk8 = np.ascontiguousarray(
        to8(WS * wkT).reshape(NC2, 2, 128, D).transpose(0, 2, 1, 3))
    wq8 = np.ascontiguousarray(
        to8(WS * wqT).reshape(NC2, 2, 128, D).transpose(0, 2, 1, 3))
    wv16 = np.ascontiguousarray(wvT.astype(f16).reshape(KC, 128, D))
    wp16 = np.ascontiguousarray(wpT.astype(f16).reshape(KC, 128, D))
    bq, bk, bv = b_qkv[0:D], b_qkv[D:2 * D], b_qkv[2 * D:3 * D]

    # cst: cols 0:128 = 8*block-diag mask, col 128 = ones
    m8 = np.zeros((128, 128), dtype=np.float32)
    m8[0:64, 0:64] = 1.0 / SCALE
    m8[64:128, 64:128] = 1.0 / SCALE
    cst = np.concatenate(
        [m8, np.ones((128, 1), dtype=np.float32),
         np.zeros((128, 3), dtype=np.float32)], axis=1).astype(f16)

    in_maps = []
    for c in range(N_CORES):
        b, s = divmod(c, 2)
        chunk = np.ascontiguousarray(x[b, s * half:(s + 1) * half, :].T)
        # x8: [NC2, 128, 2, T]; x16: [TB, 128, KC, 128]
        x8 = np.ascontiguousarray(
            to8(chunk).reshape(NC2, 2, 128, T).transpose(0, 2, 1, 3))
        x16 = np.ascontiguousarray(
            chunk.astype(f16).reshape(KC, 128, TB, 128).transpose(2, 1, 0, 3))
        in_maps.append({
            "x8d": x8, "x16d": x16,
            "wk8d": wk8, "wq8d": wq8, "wv16d": wv16, "wp16d": wp16,
            "bq": np.ascontiguousarray(bq),
            "bk": np.ascontiguousarray(WS * bk),
            "bv": np.ascontiguousarray(bv),
            "bp": np.ascontiguousarray(b_proj),
            "cst": cst,
        })

    with_bias = bool(np.any(b_qkv)) or bool(np.any(b_proj))
    nc = _get_nc(with_bias)
    global _last_in_maps
    _last_in_maps = in_maps
    res = bass_utils.run_bass_kernel_spmd(nc, in_maps, core_ids=list(range(N_CORES)))

    out = np.empty((bs, seqlen, dim), dtype=np.float32)
    for c in range(N_CORES):
        b, s = divmod(c, 2)
        out[b, s * half:(s + 1) * half, :] = res.results[c]["y"]
    return out
